# revision 25
# baseline (speedup 1.0000x reference)
"""Trainium2 Bass kernel for nn_CrossAttention (B=2, T=2048, D=1024, H=16, hd=64).

Sharding: 32 (batch, head) units over 8 cores -> each core handles 1 batch and
4 contiguous heads (core c: batch c//4, heads (c%4)*4 .. +4), grouped as two
head-pairs j in {0,1}.  Host sums the 4 partial c_proj outputs per batch and
adds bc.

Per-core dataflow (bf16 operands, D-on-partitions activations), organized as
one long ScalarE exp stream that everything else hides behind:
  qpT/kpT [128, 2, 2048] bf16 = W.T @ xT (+b), K=1024 accumulated in PSUM.
  Attention runs per (pair j, q-chunk 512) at kv-tile grain: the two heads'
  K=64 score matmuls go to PE row-groups 0/64 concurrently into a double-
  buffered [128, 2, 512] f32 S tile; one exp (free-size 1024) -> es bf16;
  av matmuls ([v_h | ones].T @ es) accumulate y+colsum into [65, 2, 512]
  f32, lagging exp by LAG kv-steps.  Each unit's trailing avs are spread
  over the next unit's first steps; y is evacuated to SBUF early so the
  normalize chain (reciprocal_approx_fast -> gpsimd partition_broadcast ->
  multiply) never blocks PSUM reuse.  PSUM: S 2x2 banks + y 2 banks + a
  2-bank "aux" slot on which projection groups and c_proj t-tiles run as
  small "filler" bundles inside the attention steps, so only 3 projection
  groups precede the exp stream and only the last c_proj tiles follow it.
"""

import sys

sys.path.insert(0, "/opt/trn_rl_repo")

from collections import deque

import numpy as np
import ml_dtypes

import concourse.bacc as bacc
import concourse.bass as bass
import concourse.mybir as mybir
import concourse.tile as tile
from concourse.bass_utils import run_bass_kernel_spmd

F32 = mybir.dt.float32
BF16 = mybir.dt.bfloat16

T = 2048          # sequence length (q and kv)
D = 1024          # model dim
HL = 4            # heads per core
HD = 64           # head dim
DH = HL * HD      # 256 local projected dim
P = 128
QC = 512          # q chunk for attention (y/norm granularity)
NU = T // QC      # 4 q-chunks
NKV = T // P      # 16 kv tiles
SCALE = 1.0 / 8.0  # 1/sqrt(64)
LAG = 10          # kv-steps by which av trails exp

N_CORES = 8

_cache = {}


def build_nc():
    if "nc" in _cache:
        return _cache["nc"]
    nc = bacc.Bacc(
        "TRN2",
        target_bir_lowering=False,
        debug=False,
        num_devices=N_CORES,
    )

    qT = nc.declare_dram_parameter("qT", [D, T], BF16, isOutput=False)
    kT = nc.declare_dram_parameter("kT", [D, T], BF16, isOutput=False)
    vext = nc.declare_dram_parameter(
        "vext", [P, HL, NKV, HD + 1], BF16, isOutput=False
    )
    Wq_r = nc.declare_dram_parameter("Wq_r", [P, 8, DH], BF16, isOutput=False)
    Wk_r = nc.declare_dram_parameter("Wk_r", [P, 8, DH], BF16, isOutput=False)
    Wc_r = nc.declare_dram_parameter("Wc_r", [P, 2, D], BF16, isOutput=False)
    bqk = nc.declare_dram_parameter("bqk", [P, 4], F32, isOutput=False)
    out = nc.declare_dram_parameter("out", [T, D], BF16, isOutput=True)

    with tile.TileContext(nc) as tc:
        with (
            tc.tile_pool(name="wpool", bufs=1) as wpool,
            tc.tile_pool(name="xpool", bufs=16) as xpool,
            tc.tile_pool(name="projsb", bufs=1) as projsb,
            tc.tile_pool(name="espool", bufs=14) as espool,
            tc.tile_pool(name="npool", bufs=2) as npool,
            tc.tile_pool(name="opool", bufs=3) as opool,
            tc.tile_pool(name="ps", bufs=1, space="PSUM") as ps,
        ):
            # ---- weights / constants (host pre-packed, contiguous DMA) ----
            bias_sb = wpool.tile([P, 4], F32, name="bias_sb")  # [bq0,bq1,bk0,bk1]
            nc.sync.dma_start(bias_sb[:], bqk.ap())
            wk_sb = wpool.tile([P, 8, DH], BF16, name="wk_sb")
            nc.sync.dma_start(wk_sb[:], Wk_r.ap())
            wq_sb = wpool.tile([P, 8, DH], BF16, name="wq_sb")
            nc.sync.dma_start(wq_sb[:], Wq_r.ap())
            wc_sb = wpool.tile([P, 2, D], BF16, name="wc_sb")
            ve_sb = wpool.tile([P, HL, NKV, HD + 1], BF16, name="ve_sb")

            # preload the exp activation table during the DMA ramp
            warm_sb = wpool.tile([P, 4], F32, name="warm_sb")
            nc.scalar.activation(
                warm_sb[:], bias_sb[:], mybir.ActivationFunctionType.Exp,
                scale=0.0,
            )

            kpT = projsb.tile([P, 2, T], BF16, name="kpT")
            qpT = projsb.tile([P, 2, T], BF16, name="qpT")
            yallT = projsb.tile([P, 2, T], BF16, name="yallT")

            # ---- input streams, split by column half and ordered so each
            # projection group's operands arrive just-in-time:
            # k cols 0:1024 -> k cols 1024:2048 -> q cols 0:1024 -> rest
            xts = {}

            def xload(nm, xd, tcp):
                for i in range(8):
                    xt = xpool.tile([P, 1024], BF16, tag=f"x{tcp}",
                                    bufs=16, name=f"x{nm}{i}{tcp}")
                    nc.sync.dma_start(
                        xt[:],
                        xd.ap()[i * P:(i + 1) * P,
                                tcp * 1024:(tcp + 1) * 1024],
                    )
                    xts[nm, i, tcp] = xt

            xload("k", kT, 0)
            xload("q", qT, 0)
            nc.sync.dma_start(ve_sb[:], vext.ap())
            nc.sync.dma_start(wc_sb[:], Wc_r.ap())
            xload("k", kT, 1)
            xload("q", qT, 1)

            projw = {"k": (wk_sb, 2, kpT), "q": (wq_sb, 0, qpT)}

            def pgroup_closures(nm, j, tcp, tag="aux", bufs=1):
                """One projection PSUM group = 16 accumulating matmuls +
                bias evac on the aux slot, chopped into <=3-matmul filler
                closures.  Lifecycles on aux are strictly sequential."""
                w_sb, bias_col0, xpT = projw[nm]
                st = {}
                mms = [(i, t2) for i in range(8) for t2 in range(2)]

                def emit(lo, hi):
                    def go():
                        if "g" not in st:
                            st["g"] = ps.tile([P, 2, 512], F32, tag=tag,
                                              bufs=bufs,
                                              name=f"pg{nm}{j}{tcp}")
                        for i, t2 in mms[lo:hi]:
                            nc.tensor.matmul(
                                st["g"][:, t2, :],
                                w_sb[:, i, j * P:(j + 1) * P],
                                xts[nm, i, tcp][:, t2 * 512:(t2 + 1) * 512],
                                start=(i == 0),
                                stop=(i == 7),
                            )
                    return go

                def evac():
                    nc.vector.tensor_tensor(
                        xpT[:, j, tcp * 1024:(tcp + 1) * 1024],
                        st["g"].rearrange("p a b -> p (a b)"),
                        bias_sb[:, bias_col0 + j:bias_col0 + j + 1]
                        .to_broadcast((P, 1024)),
                        mybir.AluOpType.add,
                    )

                cs = [emit(lo, min(lo + 3, 16)) for lo in range(0, 16, 3)]
                cs.append(evac)
                return cs

            def cp_closure(tt):
                """One c_proj t-tile on the aux slot: 4 matmuls (K=256 over
                j) + evac + DMA, as a single small filler closure."""
                def go():
                    cp = ps.tile([P, 2, 512], F32, tag="aux", name=f"cp{tt}")
                    for j in range(2):
                        for nch in range(2):
                            nc.tensor.matmul(
                                cp[:, nch, :],
                                yallT[:, j, tt * P:(tt + 1) * P],
                                wc_sb[:, j, nch * 512:(nch + 1) * 512],
                                start=(j == 0),
                                stop=(j == 1),
                            )
                    o_sb = opool.tile([P, D], BF16, tag="osb", name=f"o{tt}")
                    nc.vector.tensor_copy(
                        o_sb[:], cp.rearrange("p a b -> p (a b)")
                    )
                    nc.sync.dma_start(
                        out.ap()[tt * P:(tt + 1) * P, :], o_sb[:]
                    )
                return go

            # inline projection prologue: only what attention unit (0,0)
            # kk 0..7 needs (k j0 cols 0:1024, q j0 cols 0:1024 which covers
            # q-chunks 0 and 1).  The k group runs on the "s" slots (still
            # free) so the q group overlaps it on aux.
            for cl in (pgroup_closures("k", 0, 0, tag="s", bufs=2)
                       + pgroup_closures("q", 0, 0)):
                cl()

            # remaining projection groups stream in as filler, ordered by
            # first-consumer deadline (k01 by unit0 kk=8; k10/q10 by unit1;
            # k11 by unit1 kk=8; q01 by unit4; q11 by unit5)
            filler = deque()
            for nm, j, tcp in (("k", 0, 1), ("k", 1, 0), ("q", 1, 0),
                               ("k", 1, 1), ("q", 0, 1), ("q", 1, 1)):
                filler.extend(pgroup_closures(nm, j, tcp))

            # ---- attention units ----
            def attn_unit(j, c):
                q0 = c * QC
                st = {}
                es_tiles = {}

                def step_mm(kk):
                    s_ps = ps.tile([P, 2, 512], F32, tag="s", bufs=2,
                                   name=f"s{j}{c}{kk}")
                    for s in range(2):  # head slot: partitions s*64..s*64+64
                        p0 = s * 64
                        nc.tensor.matmul(
                            s_ps[:, s, :],
                            kpT[p0:p0 + HD, j, kk * P:(kk + 1) * P],
                            qpT[p0:p0 + HD, j, q0:q0 + QC],
                            start=True,
                            stop=True,
                        )
                    es = espool.tile([P, 2, QC], BF16, tag="es",
                                     name=f"e{j}{c}{kk}")
                    nc.scalar.activation(
                        es[:], s_ps[:], mybir.ActivationFunctionType.Exp,
                        scale=SCALE,
                    )
                    es_tiles[kk] = es

                def av_mm(kk):
                    if "y" not in st:
                        st["y"] = ps.tile([P, 2, QC], F32, tag="y",
                                          name=f"y{j}{c}")
                    for s in range(2):
                        h = 2 * j + s
                        nc.tensor.matmul(
                            st["y"][0:HD + 1, s, :],
                            ve_sb[:, h, kk, :],
                            es_tiles[kk][:, s, :],
                            start=(kk == 0),
                            stop=(kk == NKV - 1),
                        )
                    del es_tiles[kk]

                def norm():
                    # evacuate y+colsum to SBUF first (frees the PSUM slot),
                    # then normalize off-PSUM.
                    y_ps = st["y"]
                    colsum2 = npool.tile([1, 2, QC], F32, tag="colsum",
                                         name=f"cs{j}{c}")
                    nc.vector.tensor_copy(colsum2[:], y_ps[HD:HD + 1, :, :])
                    yev = npool.tile([HD, 2, QC], F32, tag="yev",
                                     name=f"ye{j}{c}")
                    nc.vector.tensor_copy(yev[:], y_ps[0:HD, :, :])
                    recip2 = npool.tile([1, 2, QC], F32, tag="recip",
                                        name=f"rc{j}{c}")
                    nc.vector.reciprocal_approx_fast(
                        out=recip2[:], in_=colsum2[:]
                    )
                    bcast2 = npool.tile([HD, 2, QC], F32, tag="bcast",
                                        name=f"bc{j}{c}")
                    nc.gpsimd.partition_broadcast(bcast2[:], recip2[:])
                    nc.vector.tensor_tensor(
                        yallT[0:HD, j, q0:q0 + QC],
                        yev[:, 0, :], bcast2[:, 0, :],
                        mybir.AluOpType.mult,
                    )
                    yn = npool.tile([HD, QC], BF16, tag="yn", name=f"yn{j}{c}")
                    nc.vector.tensor_tensor(
                        yn[:], yev[:, 1, :], bcast2[:, 1, :],
                        mybir.AluOpType.mult,
                    )
                    nc.sync.dma_start(
                        yallT[64:64 + HD, j, q0:q0 + QC], yn[:]
                    )

                return step_mm, av_mm, norm

            units = [(j, c) for c in range(NU) for j in range(2)]
            prev = None
            for u_idx, (j, c) in enumerate(units):
                step_mm, av_mm, norm = attn_unit(j, c)
                for kk in range(NKV):
                    step_mm(kk)
                    if prev is not None:
                        # spread the previous unit's 10 trailing avs over
                        # kk 0..8 (2 on kk=0), then its normalize at kk=8 —
                        # its y-slot is free well before our av(0) at kk=LAG
                        if kk == 0:
                            prev["avs"].popleft()()
                        if kk <= 8 and prev["avs"]:
                            prev["avs"].popleft()()
                        if kk == 8:
                            prev["norm"]()
                            if j == 0 and c >= 1:
                                # q-chunk c-1 rows of yallT complete
                                for tt in range(4 * (c - 1), 4 * c):
                                    filler.append(cp_closure(tt))
                    if kk >= LAG:
                        av_mm(kk - LAG)
                    # last unit: drain avs twice as fast so the post-stream
                    # tail only holds avs 11..15 + normalize
                    if u_idx == len(units) - 1 and kk >= 11:
                        av_mm(kk - 5)
                    # pops start at kk=1 so every filler group's last closure
                    # is emitted strictly before its first consumer step;
                    # unit 0 has no trailing avs, so it pops double after
                    # kk=8 to retire k10/q10 before unit 1 begins
                    if kk >= 1 and filler:
                        filler.popleft()()
                    if u_idx == 0 and kk >= 8 and filler:
                        filler.popleft()()
                first_trail = 11 if u_idx == len(units) - 1 else NKV - LAG
                prev = {
                    "avs": deque(
                        (lambda kk2=kk2, f=av_mm: f(kk2))
                        for kk2 in range(first_trail, NKV)
                    ),
                    "norm": norm,
                }
            while prev["avs"]:
                prev["avs"].popleft()()
            prev["norm"]()
            for tt in range(4 * (NU - 1), 4 * NU):
                filler.append(cp_closure(tt))
            while filler:
                filler.popleft()()

    nc.compile()
    _cache["nc"] = nc
    return nc


def make_in_maps(k, q, v, Wk, bk, Wq, bq, Wc, bc):
    bf = ml_dtypes.bfloat16
    k = np.asarray(k, dtype=np.float32)
    q = np.asarray(q, dtype=np.float32)
    v = np.asarray(v, dtype=np.float32)
    Wk = np.asarray(Wk, dtype=np.float32)
    Wq = np.asarray(Wq, dtype=np.float32)
    Wc = np.asarray(Wc, dtype=np.float32)
    bk = np.asarray(bk, dtype=np.float32)
    bq = np.asarray(bq, dtype=np.float32)
    in_maps = []
    for cidx in range(N_CORES):
        b = cidx // 4
        h0 = (cidx % 4) * HL
        sl = slice(h0 * HD, h0 * HD + DH)
        bq_t = np.ascontiguousarray(bq[sl].reshape(2, P).T)  # [128, 2]
        bk_t = np.ascontiguousarray(bk[sl].reshape(2, P).T)
        bqk = np.concatenate([bq_t, bk_t], axis=1)           # [128, 4]
        # vext [P, HL, NKV, HD+1]: [p, h, m, d] = v[m*128+p, sl][h*64+d],
        # ones at d=64 (colsum row for the av matmul)
        vsl = v[b][:, sl]                                    # [T, 256]
        ve = np.ones((P, HL, NKV, HD + 1), dtype=np.float32)
        ve[:, :, :, 0:HD] = (
            vsl.reshape(NKV, P, HL, HD).transpose(1, 2, 0, 3)
        )
        # Wq_r [128, 8, 256]: [p, i, m] = Wq[sl,:].T[i*128+p, m]
        wq_t = Wq[sl, :].T.reshape(8, P, DH).transpose(1, 0, 2)
        wk_t = Wk[sl, :].T.reshape(8, P, DH).transpose(1, 0, 2)
        wc_t = Wc[:, sl].T.reshape(2, P, D).transpose(1, 0, 2)
        in_maps.append({
            "qT": np.ascontiguousarray(q[b].T).astype(bf),
            "kT": np.ascontiguousarray(k[b].T).astype(bf),
            "vext": np.ascontiguousarray(ve).astype(bf),
            "Wq_r": np.ascontiguousarray(wq_t).astype(bf),
            "Wk_r": np.ascontiguousarray(wk_t).astype(bf),
            "Wc_r": np.ascontiguousarray(wc_t).astype(bf),
            "bqk": np.ascontiguousarray(bqk),
        })
    return in_maps


def kernel(k, q, v, Wk, bk, Wq, bq, Wc, bc, _trace=False, _trace_cores=None):
    bc = np.asarray(bc, dtype=np.float32)
    nc = build_nc()
    in_maps = make_in_maps(k, q, v, Wk, bk, Wq, bq, Wc, bc)
    res = run_bass_kernel_spmd(
        nc, in_maps, core_ids=list(range(N_CORES)),
        trace=_trace, trace_cores=_trace_cores,
    )
    outs = [res.results[c]["out"].astype(np.float32) for c in range(N_CORES)]
    full = np.stack([
        outs[0] + outs[1] + outs[2] + outs[3],
        outs[4] + outs[5] + outs[6] + outs[7],
    ]) + bc[None, None, :]
    kernel.last_result = res
    return full.astype(np.float32)


# revision 28
# speedup vs baseline: 1.0639x; 1.0639x over previous
"""Trainium2 Bass kernel for nn_CrossAttention (B=2, T=2048, D=1024, H=16, hd=64).

Sharding: 32 (batch, head) units over 8 cores -> each core handles 1 batch and
4 contiguous heads (core c: batch c//4, heads (c%4)*4 .. +4), grouped as two
head-pairs j in {0,1}.  Host sums the 4 partial c_proj outputs per batch and
adds bc.

Per-core dataflow (bf16 operands, D-on-partitions activations), organized as
one long ScalarE exp stream that everything else hides behind:
  qpT/kpT [128, 2, 2048] bf16 = W.T @ xT (+b), K=1024 accumulated in PSUM.
  Attention runs per (pair j, q-chunk 512) at kv-tile grain: the two heads'
  K=64 score matmuls go to PE row-groups 0/64 concurrently into a double-
  buffered [128, 2, 512] f32 S tile; one exp (free-size 1024) -> es bf16;
  av matmuls ([v_h | ones].T @ es) accumulate y+colsum into [65, 2, 512]
  f32, lagging exp by LAG kv-steps.  Each unit's trailing avs are spread
  over the next unit's first steps; y is evacuated to SBUF early so the
  normalize chain (reciprocal_approx_fast -> gpsimd partition_broadcast ->
  multiply) never blocks PSUM reuse.  PSUM: S 2x2 banks + y 2 banks + a
  2-bank "aux" slot on which projection groups and c_proj t-tiles run as
  small "filler" bundles inside the attention steps, so only 3 projection
  groups precede the exp stream and only the last c_proj tiles follow it.
"""

import sys

sys.path.insert(0, "/opt/trn_rl_repo")

from collections import deque

import numpy as np
import ml_dtypes

import concourse.bacc as bacc
import concourse.bass as bass
import concourse.mybir as mybir
import concourse.tile as tile
from concourse.bass_utils import run_bass_kernel_spmd

F32 = mybir.dt.float32
BF16 = mybir.dt.bfloat16

T = 2048          # sequence length (q and kv)
D = 1024          # model dim
HL = 4            # heads per core
HD = 64           # head dim
DH = HL * HD      # 256 local projected dim
P = 128
QC = 512          # q chunk for attention (y/norm granularity)
NU = T // QC      # 4 q-chunks
NKV = T // P      # 16 kv tiles
SCALE = 1.0 / 8.0  # 1/sqrt(64)
LAG = 10          # kv-steps by which av trails exp

N_CORES = 8

_cache = {}


def build_nc():
    if "nc" in _cache:
        return _cache["nc"]
    nc = bacc.Bacc(
        "TRN2",
        target_bir_lowering=False,
        debug=False,
        num_devices=N_CORES,
    )

    qT = nc.declare_dram_parameter("qT", [D, T], BF16, isOutput=False)
    kT = nc.declare_dram_parameter("kT", [D, T], BF16, isOutput=False)
    vext = nc.declare_dram_parameter(
        "vext", [P, HL, NKV, HD + 1], BF16, isOutput=False
    )
    Wq_r = nc.declare_dram_parameter("Wq_r", [P, 8, DH], BF16, isOutput=False)
    Wk_r = nc.declare_dram_parameter("Wk_r", [P, 8, DH], BF16, isOutput=False)
    Wc_r = nc.declare_dram_parameter("Wc_r", [P, 2, D], BF16, isOutput=False)
    bqk = nc.declare_dram_parameter("bqk", [P, 4], F32, isOutput=False)
    out = nc.declare_dram_parameter("out", [T, D], BF16, isOutput=True)

    with tile.TileContext(nc) as tc:
        with (
            tc.tile_pool(name="wpool", bufs=1) as wpool,
            tc.tile_pool(name="xpool", bufs=16) as xpool,
            tc.tile_pool(name="projsb", bufs=1) as projsb,
            tc.tile_pool(name="espool", bufs=14) as espool,
            tc.tile_pool(name="npool", bufs=2) as npool,
            tc.tile_pool(name="opool", bufs=3) as opool,
            tc.tile_pool(name="ps", bufs=1, space="PSUM") as ps,
        ):
            # ---- weights / constants (host pre-packed, contiguous DMA) ----
            bias_sb = wpool.tile([P, 4], F32, name="bias_sb")  # [bq0,bq1,bk0,bk1]
            nc.sync.dma_start(bias_sb[:], bqk.ap())
            wk_sb = wpool.tile([P, 8, DH], BF16, name="wk_sb")
            nc.sync.dma_start(wk_sb[:], Wk_r.ap())
            wq_sb = wpool.tile([P, 8, DH], BF16, name="wq_sb")
            nc.sync.dma_start(wq_sb[:], Wq_r.ap())
            wc_sb = wpool.tile([P, 2, D], BF16, name="wc_sb")
            ve_sb = wpool.tile([P, HL, NKV, HD + 1], BF16, name="ve_sb")

            # preload the exp activation table during the DMA ramp
            warm_sb = wpool.tile([P, 4], F32, name="warm_sb")
            nc.scalar.activation(
                warm_sb[:], bias_sb[:], mybir.ActivationFunctionType.Exp,
                scale=0.0,
            )

            kpT = projsb.tile([P, 2, T], BF16, name="kpT")
            qpT = projsb.tile([P, 2, T], BF16, name="qpT")
            yallT = projsb.tile([P, 2, T], BF16, name="yallT")

            # ---- input streams, split by column half and ordered so each
            # projection group's operands arrive just-in-time:
            # k cols 0:1024 -> k cols 1024:2048 -> q cols 0:1024 -> rest
            xts = {}

            def xload(nm, xd, tcp):
                for i in range(8):
                    xt = xpool.tile([P, 1024], BF16, tag=f"x{tcp}",
                                    bufs=16, name=f"x{nm}{i}{tcp}")
                    nc.sync.dma_start(
                        xt[:],
                        xd.ap()[i * P:(i + 1) * P,
                                tcp * 1024:(tcp + 1) * 1024],
                    )
                    xts[nm, i, tcp] = xt

            xload("k", kT, 0)
            xload("q", qT, 0)
            nc.sync.dma_start(ve_sb[:], vext.ap())
            nc.sync.dma_start(wc_sb[:], Wc_r.ap())
            xload("k", kT, 1)
            xload("q", qT, 1)

            projw = {"k": (wk_sb, 2, kpT), "q": (wq_sb, 0, qpT)}

            def pgroup_closures(nm, j, tcp, tag="aux", bufs=1):
                """One projection PSUM group = 16 accumulating matmuls +
                bias evac on the aux slot, chopped into <=3-matmul filler
                closures.  Lifecycles on aux are strictly sequential."""
                w_sb, bias_col0, xpT = projw[nm]
                st = {}
                mms = [(i, t2) for i in range(8) for t2 in range(2)]

                def emit(lo, hi):
                    def go():
                        if "g" not in st:
                            st["g"] = ps.tile([P, 2, 512], F32, tag=tag,
                                              bufs=bufs,
                                              name=f"pg{nm}{j}{tcp}")
                        for i, t2 in mms[lo:hi]:
                            nc.tensor.matmul(
                                st["g"][:, t2, :],
                                w_sb[:, i, j * P:(j + 1) * P],
                                xts[nm, i, tcp][:, t2 * 512:(t2 + 1) * 512],
                                start=(i == 0),
                                stop=(i == 7),
                            )
                    return go

                def evac():
                    nc.vector.tensor_tensor(
                        xpT[:, j, tcp * 1024:(tcp + 1) * 1024],
                        st["g"].rearrange("p a b -> p (a b)"),
                        bias_sb[:, bias_col0 + j:bias_col0 + j + 1]
                        .to_broadcast((P, 1024)),
                        mybir.AluOpType.add,
                    )

                cs = [emit(lo, min(lo + 3, 16)) for lo in range(0, 16, 3)]
                cs.append(evac)
                return cs

            def cp_closure(tt):
                """One c_proj t-tile on the aux slot: 4 matmuls (K=256 over
                j) + evac + DMA, as a single small filler closure."""
                def go():
                    cp = ps.tile([P, 2, 512], F32, tag="aux", name=f"cp{tt}")
                    for j in range(2):
                        for nch in range(2):
                            nc.tensor.matmul(
                                cp[:, nch, :],
                                yallT[:, j, tt * P:(tt + 1) * P],
                                wc_sb[:, j, nch * 512:(nch + 1) * 512],
                                start=(j == 0),
                                stop=(j == 1),
                            )
                    o_sb = opool.tile([P, D], BF16, tag="osb", name=f"o{tt}")
                    nc.vector.tensor_copy(
                        o_sb[:], cp.rearrange("p a b -> p (a b)")
                    )
                    nc.sync.dma_start(
                        out.ap()[tt * P:(tt + 1) * P, :], o_sb[:]
                    )
                return go

            # inline projection prologue: only what attention unit (0,0)
            # kk 0..7 needs (k j0 cols 0:1024, q j0 cols 0:1024 which covers
            # q-chunks 0 and 1).  The k group runs on the "s" slots (still
            # free) so the q group overlaps it on aux.
            for cl in (pgroup_closures("k", 0, 0, tag="s", bufs=2)
                       + pgroup_closures("q", 0, 0)):
                cl()

            # remaining projection groups stream in as filler, ordered by
            # first-consumer deadline (k01 by unit0 kk=8; k10/q10 by unit1;
            # k11 by unit1 kk=8; q01 by unit4; q11 by unit5)
            filler = deque()
            for nm, j, tcp in (("k", 0, 1), ("k", 1, 0), ("q", 1, 0),
                               ("k", 1, 1), ("q", 0, 1), ("q", 1, 1)):
                filler.extend(pgroup_closures(nm, j, tcp))

            # ---- attention units ----
            def attn_unit(j, c, last=False):
                q0 = c * QC
                st = {}
                es_tiles = {}

                def step_mm(kk):
                    s_ps = ps.tile([P, 2, 512], F32, tag="s", bufs=2,
                                   name=f"s{j}{c}{kk}")
                    for s in range(2):  # head slot: partitions s*64..s*64+64
                        p0 = s * 64
                        nc.tensor.matmul(
                            s_ps[:, s, :],
                            kpT[p0:p0 + HD, j, kk * P:(kk + 1) * P],
                            qpT[p0:p0 + HD, j, q0:q0 + QC],
                            start=True,
                            stop=True,
                        )
                    es = espool.tile([P, 2, QC], BF16, tag="es",
                                     name=f"e{j}{c}{kk}")
                    nc.scalar.activation(
                        es[:], s_ps[:], mybir.ActivationFunctionType.Exp,
                        scale=SCALE,
                    )
                    es_tiles[kk] = es

                def av_mm(kk):
                    if "y" not in st:
                        st["y"] = ps.tile([P, 2, QC], F32, tag="y",
                                          name=f"y{j}{c}")
                    for s in range(2):
                        h = 2 * j + s
                        nc.tensor.matmul(
                            st["y"][0:HD + 1, s, :],
                            ve_sb[:, h, kk, :],
                            es_tiles[kk][:, s, :],
                            start=(kk == 0),
                            stop=(kk == NKV - 1),
                        )
                    del es_tiles[kk]

                def norm():
                    # evacuate y+colsum to SBUF first (frees the PSUM slot),
                    # then normalize off-PSUM.  The final unit's y slot has
                    # no successor, so it skips the y evacuation and its
                    # multiplies read PSUM directly (shorter tail chain).
                    y_ps = st["y"]
                    colsum2 = npool.tile([1, 2, QC], F32, tag="colsum",
                                         name=f"cs{j}{c}")
                    nc.vector.tensor_copy(colsum2[:], y_ps[HD:HD + 1, :, :])
                    if last:
                        yev = y_ps
                    else:
                        yev = npool.tile([HD, 2, QC], F32, tag="yev",
                                         name=f"ye{j}{c}")
                        nc.vector.tensor_copy(yev[:], y_ps[0:HD, :, :])
                    recip2 = npool.tile([1, 2, QC], F32, tag="recip",
                                        name=f"rc{j}{c}")
                    nc.vector.reciprocal_approx_fast(
                        out=recip2[:], in_=colsum2[:]
                    )
                    bcast2 = npool.tile([HD, 2, QC], F32, tag="bcast",
                                        name=f"bc{j}{c}")
                    nc.gpsimd.partition_broadcast(bcast2[:], recip2[:])
                    nc.vector.tensor_tensor(
                        yallT[0:HD, j, q0:q0 + QC],
                        yev[0:HD, 0, :], bcast2[:, 0, :],
                        mybir.AluOpType.mult,
                    )
                    yn = npool.tile([HD, QC], BF16, tag="yn", name=f"yn{j}{c}")
                    nc.vector.tensor_tensor(
                        yn[:], yev[0:HD, 1, :], bcast2[:, 1, :],
                        mybir.AluOpType.mult,
                    )
                    nc.sync.dma_start(
                        yallT[64:64 + HD, j, q0:q0 + QC], yn[:]
                    )

                return step_mm, av_mm, norm

            units = [(j, c) for c in range(NU) for j in range(2)]
            prev = None
            for u_idx, (j, c) in enumerate(units):
                step_mm, av_mm, norm = attn_unit(
                    j, c, last=(u_idx == len(units) - 1)
                )
                for kk in range(NKV):
                    step_mm(kk)
                    if prev is not None:
                        # spread the previous unit's 10 trailing avs over
                        # kk 0..7 (1 each) + the last two with its normalize
                        # at kk=8 — its y-slot is free before av(0) at LAG
                        if kk <= 7 and prev["avs"]:
                            prev["avs"].popleft()()
                        if kk == 8:
                            while prev["avs"]:
                                prev["avs"].popleft()()
                            prev["norm"]()
                            if j == 0 and c >= 1:
                                # q-chunk c-1 rows of yallT complete
                                for tt in range(4 * (c - 1), 4 * c):
                                    filler.append(cp_closure(tt))
                    if kk >= LAG:
                        av_mm(kk - LAG)
                    # last unit: drain avs twice as fast so the post-stream
                    # tail only holds avs 11..15 + normalize
                    if u_idx == len(units) - 1 and kk >= 11:
                        av_mm(kk - 5)
                    # pops start at kk=1 so every filler group's last closure
                    # is emitted strictly before its first consumer step;
                    # unit 0 carries no av work before kk=10, so it pops
                    # double from the start to retire k01/k10/q10 early
                    if kk >= 1 and filler:
                        filler.popleft()()
                    if u_idx == 0 and kk >= 1 and filler:
                        filler.popleft()()
                first_trail = 11 if u_idx == len(units) - 1 else NKV - LAG
                prev = {
                    "avs": deque(
                        (lambda kk2=kk2, f=av_mm: f(kk2))
                        for kk2 in range(first_trail, NKV)
                    ),
                    "norm": norm,
                }
            while prev["avs"]:
                prev["avs"].popleft()()
            prev["norm"]()
            for tt in range(4 * (NU - 1), 4 * NU):
                filler.append(cp_closure(tt))
            while filler:
                filler.popleft()()

    nc.compile()
    _cache["nc"] = nc
    return nc


def make_in_maps(k, q, v, Wk, bk, Wq, bq, Wc, bc):
    bf = ml_dtypes.bfloat16
    k = np.asarray(k, dtype=np.float32)
    q = np.asarray(q, dtype=np.float32)
    v = np.asarray(v, dtype=np.float32)
    Wk = np.asarray(Wk, dtype=np.float32)
    Wq = np.asarray(Wq, dtype=np.float32)
    Wc = np.asarray(Wc, dtype=np.float32)
    bk = np.asarray(bk, dtype=np.float32)
    bq = np.asarray(bq, dtype=np.float32)
    in_maps = []
    for cidx in range(N_CORES):
        b = cidx // 4
        h0 = (cidx % 4) * HL
        sl = slice(h0 * HD, h0 * HD + DH)
        bq_t = np.ascontiguousarray(bq[sl].reshape(2, P).T)  # [128, 2]
        bk_t = np.ascontiguousarray(bk[sl].reshape(2, P).T)
        bqk = np.concatenate([bq_t, bk_t], axis=1)           # [128, 4]
        # vext [P, HL, NKV, HD+1]: [p, h, m, d] = v[m*128+p, sl][h*64+d],
        # ones at d=64 (colsum row for the av matmul)
        vsl = v[b][:, sl]                                    # [T, 256]
        ve = np.ones((P, HL, NKV, HD + 1), dtype=np.float32)
        ve[:, :, :, 0:HD] = (
            vsl.reshape(NKV, P, HL, HD).transpose(1, 2, 0, 3)
        )
        # Wq_r [128, 8, 256]: [p, i, m] = Wq[sl,:].T[i*128+p, m]
        wq_t = Wq[sl, :].T.reshape(8, P, DH).transpose(1, 0, 2)
        wk_t = Wk[sl, :].T.reshape(8, P, DH).transpose(1, 0, 2)
        wc_t = Wc[:, sl].T.reshape(2, P, D).transpose(1, 0, 2)
        in_maps.append({
            "qT": np.ascontiguousarray(q[b].T).astype(bf),
            "kT": np.ascontiguousarray(k[b].T).astype(bf),
            "vext": np.ascontiguousarray(ve).astype(bf),
            "Wq_r": np.ascontiguousarray(wq_t).astype(bf),
            "Wk_r": np.ascontiguousarray(wk_t).astype(bf),
            "Wc_r": np.ascontiguousarray(wc_t).astype(bf),
            "bqk": np.ascontiguousarray(bqk),
        })
    return in_maps


def kernel(k, q, v, Wk, bk, Wq, bq, Wc, bc, _trace=False, _trace_cores=None):
    bc = np.asarray(bc, dtype=np.float32)
    nc = build_nc()
    in_maps = make_in_maps(k, q, v, Wk, bk, Wq, bq, Wc, bc)
    res = run_bass_kernel_spmd(
        nc, in_maps, core_ids=list(range(N_CORES)),
        trace=_trace, trace_cores=_trace_cores,
    )
    outs = [res.results[c]["out"].astype(np.float32) for c in range(N_CORES)]
    full = np.stack([
        outs[0] + outs[1] + outs[2] + outs[3],
        outs[4] + outs[5] + outs[6] + outs[7],
    ]) + bc[None, None, :]
    kernel.last_result = res
    return full.astype(np.float32)


# revision 29
# speedup vs baseline: 1.0700x; 1.0057x over previous
"""Trainium2 Bass kernel for nn_CrossAttention (B=2, T=2048, D=1024, H=16, hd=64).

Sharding: 32 (batch, head) units over 8 cores -> each core handles 1 batch and
4 contiguous heads (core c: batch c//4, heads (c%4)*4 .. +4), grouped as two
head-pairs j in {0,1}.  Host sums the 4 partial c_proj outputs per batch and
adds bc.

Per-core dataflow (bf16 operands, D-on-partitions activations), organized as
one long ScalarE exp stream that everything else hides behind:
  qpT/kpT [128, 2, 2048] bf16 = W.T @ xT (+b), K=1024 accumulated in PSUM.
  Attention runs per (pair j, q-chunk 512) at kv-tile grain: the two heads'
  K=64 score matmuls go to PE row-groups 0/64 concurrently into a double-
  buffered [128, 2, 512] f32 S tile; one exp (free-size 1024) -> es bf16;
  av matmuls ([v_h | ones].T @ es) accumulate y+colsum into [65, 2, 512]
  f32, lagging exp by LAG kv-steps.  Each unit's trailing avs are spread
  over the next unit's first steps; y is evacuated to SBUF early so the
  normalize chain (reciprocal_approx_fast -> gpsimd partition_broadcast ->
  multiply) never blocks PSUM reuse.  PSUM: S 2x2 banks + y 2 banks + a
  2-bank "aux" slot on which projection groups and c_proj t-tiles run as
  small "filler" bundles inside the attention steps, so only 3 projection
  groups precede the exp stream and only the last c_proj tiles follow it.
"""

import sys

sys.path.insert(0, "/opt/trn_rl_repo")

from collections import deque

import numpy as np
import ml_dtypes

import concourse.bacc as bacc
import concourse.bass as bass
import concourse.mybir as mybir
import concourse.tile as tile
from concourse.bass_utils import run_bass_kernel_spmd

F32 = mybir.dt.float32
BF16 = mybir.dt.bfloat16

T = 2048          # sequence length (q and kv)
D = 1024          # model dim
HL = 4            # heads per core
HD = 64           # head dim
DH = HL * HD      # 256 local projected dim
P = 128
QC = 512          # q chunk for attention (y/norm granularity)
NU = T // QC      # 4 q-chunks
NKV = T // P      # 16 kv tiles
SCALE = 1.0 / 8.0  # 1/sqrt(64)
LAG = 10          # kv-steps by which av trails exp

N_CORES = 8

_cache = {}


def build_nc():
    if "nc" in _cache:
        return _cache["nc"]
    nc = bacc.Bacc(
        "TRN2",
        target_bir_lowering=False,
        debug=False,
        num_devices=N_CORES,
    )

    qT = nc.declare_dram_parameter("qT", [D, T], BF16, isOutput=False)
    kT = nc.declare_dram_parameter("kT", [D, T], BF16, isOutput=False)
    vext = nc.declare_dram_parameter(
        "vext", [P, HL, NKV, HD + 1], BF16, isOutput=False
    )
    Wq_r = nc.declare_dram_parameter("Wq_r", [P, 8, DH], BF16, isOutput=False)
    Wk_r = nc.declare_dram_parameter("Wk_r", [P, 8, DH], BF16, isOutput=False)
    Wc_r = nc.declare_dram_parameter("Wc_r", [P, 2, D], BF16, isOutput=False)
    bqk = nc.declare_dram_parameter("bqk", [P, 4], F32, isOutput=False)
    out = nc.declare_dram_parameter("out", [T, D], BF16, isOutput=True)

    with tile.TileContext(nc) as tc:
        with (
            tc.tile_pool(name="wpool", bufs=1) as wpool,
            tc.tile_pool(name="xpool", bufs=16) as xpool,
            tc.tile_pool(name="projsb", bufs=1) as projsb,
            tc.tile_pool(name="espool", bufs=14) as espool,
            tc.tile_pool(name="npool", bufs=2) as npool,
            tc.tile_pool(name="opool", bufs=3) as opool,
            tc.tile_pool(name="ps", bufs=1, space="PSUM") as ps,
        ):
            # ---- weights / constants (host pre-packed, contiguous DMA) ----
            bias_sb = wpool.tile([P, 4], F32, name="bias_sb")  # [bq0,bq1,bk0,bk1]
            nc.sync.dma_start(bias_sb[:], bqk.ap())
            wk_sb = wpool.tile([P, 8, DH], BF16, name="wk_sb")
            nc.sync.dma_start(wk_sb[:], Wk_r.ap())
            wq_sb = wpool.tile([P, 8, DH], BF16, name="wq_sb")
            nc.sync.dma_start(wq_sb[:], Wq_r.ap())
            wc_sb = wpool.tile([P, 2, D], BF16, name="wc_sb")
            ve_sb = wpool.tile([P, HL, NKV, HD + 1], BF16, name="ve_sb")

            # preload the exp activation table during the DMA ramp
            warm_sb = wpool.tile([P, 4], F32, name="warm_sb")
            nc.scalar.activation(
                warm_sb[:], bias_sb[:], mybir.ActivationFunctionType.Exp,
                scale=0.0,
            )

            kpT = projsb.tile([P, 2, T], BF16, name="kpT")
            qpT = projsb.tile([P, 2, T], BF16, name="qpT")
            yallT = projsb.tile([P, 2, T], BF16, name="yallT")

            # ---- input streams, split by column half and ordered so each
            # projection group's operands arrive just-in-time:
            # k cols 0:1024 -> k cols 1024:2048 -> q cols 0:1024 -> rest
            xts = {}

            def xload(nm, xd, tcp):
                for i in range(8):
                    xt = xpool.tile([P, 1024], BF16, tag=f"x{tcp}",
                                    bufs=16, name=f"x{nm}{i}{tcp}")
                    nc.sync.dma_start(
                        xt[:],
                        xd.ap()[i * P:(i + 1) * P,
                                tcp * 1024:(tcp + 1) * 1024],
                    )
                    xts[nm, i, tcp] = xt

            xload("k", kT, 0)
            xload("q", qT, 0)
            xload("k", kT, 1)
            nc.sync.dma_start(ve_sb[:], vext.ap())
            nc.sync.dma_start(wc_sb[:], Wc_r.ap())
            xload("q", qT, 1)

            projw = {"k": (wk_sb, 2, kpT), "q": (wq_sb, 0, qpT)}

            def pgroup_closures(nm, j, tcp, tag="aux", bufs=1):
                """One projection PSUM group = 16 accumulating matmuls +
                bias evac on the aux slot, chopped into <=3-matmul filler
                closures.  Lifecycles on aux are strictly sequential."""
                w_sb, bias_col0, xpT = projw[nm]
                st = {}
                mms = [(i, t2) for i in range(8) for t2 in range(2)]

                def emit(lo, hi):
                    def go():
                        if "g" not in st:
                            st["g"] = ps.tile([P, 2, 512], F32, tag=tag,
                                              bufs=bufs,
                                              name=f"pg{nm}{j}{tcp}")
                        for i, t2 in mms[lo:hi]:
                            nc.tensor.matmul(
                                st["g"][:, t2, :],
                                w_sb[:, i, j * P:(j + 1) * P],
                                xts[nm, i, tcp][:, t2 * 512:(t2 + 1) * 512],
                                start=(i == 0),
                                stop=(i == 7),
                            )
                    return go

                def evac():
                    nc.vector.tensor_tensor(
                        xpT[:, j, tcp * 1024:(tcp + 1) * 1024],
                        st["g"].rearrange("p a b -> p (a b)"),
                        bias_sb[:, bias_col0 + j:bias_col0 + j + 1]
                        .to_broadcast((P, 1024)),
                        mybir.AluOpType.add,
                    )

                cs = [emit(lo, min(lo + 3, 16)) for lo in range(0, 16, 3)]
                cs.append(evac)
                return cs

            def cp_closure(tt):
                """One c_proj t-tile on the aux slot: 4 matmuls (K=256 over
                j) + evac + DMA, as a single small filler closure."""
                def go():
                    cp = ps.tile([P, 2, 512], F32, tag="aux", name=f"cp{tt}")
                    for j in range(2):
                        for nch in range(2):
                            nc.tensor.matmul(
                                cp[:, nch, :],
                                yallT[:, j, tt * P:(tt + 1) * P],
                                wc_sb[:, j, nch * 512:(nch + 1) * 512],
                                start=(j == 0),
                                stop=(j == 1),
                            )
                    o_sb = opool.tile([P, D], BF16, tag="osb", name=f"o{tt}")
                    nc.vector.tensor_copy(
                        o_sb[:], cp.rearrange("p a b -> p (a b)")
                    )
                    nc.sync.dma_start(
                        out.ap()[tt * P:(tt + 1) * P, :], o_sb[:]
                    )
                return go

            # inline projection prologue: only what attention unit (0,0)
            # kk 0..7 needs (k j0 cols 0:1024, q j0 cols 0:1024 which covers
            # q-chunks 0 and 1).  The k group runs on the "s" slots (still
            # free) so the q group overlaps it on aux.
            for cl in (pgroup_closures("k", 0, 0, tag="s", bufs=2)
                       + pgroup_closures("q", 0, 0)):
                cl()

            # remaining projection groups stream in as filler, ordered by
            # first-consumer deadline (k01 by unit0 kk=8; k10/q10 by unit1;
            # k11 by unit1 kk=8; q01 by unit4; q11 by unit5)
            filler = deque()
            for nm, j, tcp in (("k", 0, 1), ("k", 1, 0), ("q", 1, 0),
                               ("k", 1, 1), ("q", 0, 1), ("q", 1, 1)):
                filler.extend(pgroup_closures(nm, j, tcp))

            # ---- attention units ----
            def attn_unit(j, c, last=False):
                q0 = c * QC
                st = {}
                es_tiles = {}

                def step_mm(kk):
                    s_ps = ps.tile([P, 2, 512], F32, tag="s", bufs=2,
                                   name=f"s{j}{c}{kk}")
                    for s in range(2):  # head slot: partitions s*64..s*64+64
                        p0 = s * 64
                        nc.tensor.matmul(
                            s_ps[:, s, :],
                            kpT[p0:p0 + HD, j, kk * P:(kk + 1) * P],
                            qpT[p0:p0 + HD, j, q0:q0 + QC],
                            start=True,
                            stop=True,
                        )
                    es = espool.tile([P, 2, QC], BF16, tag="es",
                                     name=f"e{j}{c}{kk}")
                    nc.scalar.activation(
                        es[:], s_ps[:], mybir.ActivationFunctionType.Exp,
                        scale=SCALE,
                    )
                    es_tiles[kk] = es

                def av_mm(kk):
                    if "y" not in st:
                        st["y"] = ps.tile([P, 2, QC], F32, tag="y",
                                          name=f"y{j}{c}")
                    for s in range(2):
                        h = 2 * j + s
                        nc.tensor.matmul(
                            st["y"][0:HD + 1, s, :],
                            ve_sb[:, h, kk, :],
                            es_tiles[kk][:, s, :],
                            start=(kk == 0),
                            stop=(kk == NKV - 1),
                        )
                    del es_tiles[kk]

                def norm():
                    # evacuate y+colsum to SBUF first (frees the PSUM slot),
                    # then normalize off-PSUM.  The final unit's y slot has
                    # no successor, so it skips the y evacuation and its
                    # multiplies read PSUM directly (shorter tail chain).
                    y_ps = st["y"]
                    colsum2 = npool.tile([1, 2, QC], F32, tag="colsum",
                                         name=f"cs{j}{c}")
                    nc.vector.tensor_copy(colsum2[:], y_ps[HD:HD + 1, :, :])
                    if last:
                        yev = y_ps
                    else:
                        yev = npool.tile([HD, 2, QC], F32, tag="yev",
                                         name=f"ye{j}{c}")
                        nc.vector.tensor_copy(yev[:], y_ps[0:HD, :, :])
                    recip2 = npool.tile([1, 2, QC], F32, tag="recip",
                                        name=f"rc{j}{c}")
                    nc.vector.reciprocal_approx_fast(
                        out=recip2[:], in_=colsum2[:]
                    )
                    bcast2 = npool.tile([HD, 2, QC], F32, tag="bcast",
                                        name=f"bc{j}{c}")
                    nc.gpsimd.partition_broadcast(bcast2[:], recip2[:])
                    nc.vector.tensor_tensor(
                        yallT[0:HD, j, q0:q0 + QC],
                        yev[0:HD, 0, :], bcast2[:, 0, :],
                        mybir.AluOpType.mult,
                    )
                    yn = npool.tile([HD, QC], BF16, tag="yn", name=f"yn{j}{c}")
                    nc.vector.tensor_tensor(
                        yn[:], yev[0:HD, 1, :], bcast2[:, 1, :],
                        mybir.AluOpType.mult,
                    )
                    nc.sync.dma_start(
                        yallT[64:64 + HD, j, q0:q0 + QC], yn[:]
                    )

                return step_mm, av_mm, norm

            units = [(j, c) for c in range(NU) for j in range(2)]
            prev = None
            for u_idx, (j, c) in enumerate(units):
                step_mm, av_mm, norm = attn_unit(
                    j, c, last=(u_idx == len(units) - 1)
                )
                for kk in range(NKV):
                    step_mm(kk)
                    if prev is not None:
                        # spread the previous unit's 10 trailing avs over
                        # kk 0..7 (1 each) + the last two with its normalize
                        # at kk=8 — its y-slot is free before av(0) at LAG
                        if kk <= 7 and prev["avs"]:
                            prev["avs"].popleft()()
                        if kk == 8:
                            while prev["avs"]:
                                prev["avs"].popleft()()
                            prev["norm"]()
                            if j == 0 and c >= 1:
                                # q-chunk c-1 rows of yallT complete
                                for tt in range(4 * (c - 1), 4 * c):
                                    filler.append(cp_closure(tt))
                    if kk >= LAG:
                        av_mm(kk - LAG)
                    # last unit: drain avs twice as fast so the post-stream
                    # tail only holds avs 11..15 + normalize
                    if u_idx == len(units) - 1 and kk >= 11:
                        av_mm(kk - 5)
                    # pops start at kk=1 so every filler group's last closure
                    # is emitted strictly before its first consumer step;
                    # unit 0 carries no av work before kk=10, so it pops
                    # double from the start to retire k01/k10/q10 early
                    if kk >= 1 and filler:
                        filler.popleft()()
                    if u_idx == 0 and kk >= 1 and filler:
                        filler.popleft()()
                first_trail = 11 if u_idx == len(units) - 1 else NKV - LAG
                prev = {
                    "avs": deque(
                        (lambda kk2=kk2, f=av_mm: f(kk2))
                        for kk2 in range(first_trail, NKV)
                    ),
                    "norm": norm,
                }
            while prev["avs"]:
                prev["avs"].popleft()()
            prev["norm"]()
            for tt in range(4 * (NU - 1), 4 * NU):
                filler.append(cp_closure(tt))
            while filler:
                filler.popleft()()

    nc.compile()
    _cache["nc"] = nc
    return nc


def make_in_maps(k, q, v, Wk, bk, Wq, bq, Wc, bc):
    bf = ml_dtypes.bfloat16
    k = np.asarray(k, dtype=np.float32)
    q = np.asarray(q, dtype=np.float32)
    v = np.asarray(v, dtype=np.float32)
    Wk = np.asarray(Wk, dtype=np.float32)
    Wq = np.asarray(Wq, dtype=np.float32)
    Wc = np.asarray(Wc, dtype=np.float32)
    bk = np.asarray(bk, dtype=np.float32)
    bq = np.asarray(bq, dtype=np.float32)
    in_maps = []
    for cidx in range(N_CORES):
        b = cidx // 4
        h0 = (cidx % 4) * HL
        sl = slice(h0 * HD, h0 * HD + DH)
        bq_t = np.ascontiguousarray(bq[sl].reshape(2, P).T)  # [128, 2]
        bk_t = np.ascontiguousarray(bk[sl].reshape(2, P).T)
        bqk = np.concatenate([bq_t, bk_t], axis=1)           # [128, 4]
        # vext [P, HL, NKV, HD+1]: [p, h, m, d] = v[m*128+p, sl][h*64+d],
        # ones at d=64 (colsum row for the av matmul)
        vsl = v[b][:, sl]                                    # [T, 256]
        ve = np.ones((P, HL, NKV, HD + 1), dtype=np.float32)
        ve[:, :, :, 0:HD] = (
            vsl.reshape(NKV, P, HL, HD).transpose(1, 2, 0, 3)
        )
        # Wq_r [128, 8, 256]: [p, i, m] = Wq[sl,:].T[i*128+p, m]
        wq_t = Wq[sl, :].T.reshape(8, P, DH).transpose(1, 0, 2)
        wk_t = Wk[sl, :].T.reshape(8, P, DH).transpose(1, 0, 2)
        wc_t = Wc[:, sl].T.reshape(2, P, D).transpose(1, 0, 2)
        in_maps.append({
            "qT": np.ascontiguousarray(q[b].T).astype(bf),
            "kT": np.ascontiguousarray(k[b].T).astype(bf),
            "vext": np.ascontiguousarray(ve).astype(bf),
            "Wq_r": np.ascontiguousarray(wq_t).astype(bf),
            "Wk_r": np.ascontiguousarray(wk_t).astype(bf),
            "Wc_r": np.ascontiguousarray(wc_t).astype(bf),
            "bqk": np.ascontiguousarray(bqk),
        })
    return in_maps


def kernel(k, q, v, Wk, bk, Wq, bq, Wc, bc, _trace=False, _trace_cores=None):
    bc = np.asarray(bc, dtype=np.float32)
    nc = build_nc()
    in_maps = make_in_maps(k, q, v, Wk, bk, Wq, bq, Wc, bc)
    res = run_bass_kernel_spmd(
        nc, in_maps, core_ids=list(range(N_CORES)),
        trace=_trace, trace_cores=_trace_cores,
    )
    outs = [res.results[c]["out"].astype(np.float32) for c in range(N_CORES)]
    full = np.stack([
        outs[0] + outs[1] + outs[2] + outs[3],
        outs[4] + outs[5] + outs[6] + outs[7],
    ]) + bc[None, None, :]
    kernel.last_result = res
    return full.astype(np.float32)


# revision 33
# speedup vs baseline: 1.0876x; 1.0165x over previous
"""Trainium2 Bass kernel for nn_CrossAttention (B=2, T=2048, D=1024, H=16, hd=64).

Sharding: 32 (batch, head) units over 8 cores -> each core handles 1 batch and
4 contiguous heads (core c: batch c//4, heads (c%4)*4 .. +4), grouped as two
head-pairs j in {0,1}.  Host sums the 4 partial c_proj outputs per batch and
adds bc.

Per-core dataflow (bf16 operands, D-on-partitions activations), organized as
one long ScalarE exp stream that everything else hides behind:
  qpT/kpT [128, 2, 2048] bf16 = W.T @ xT (+b), K=1024 accumulated in PSUM.
  Attention runs per (pair j, q-chunk 512) at kv-tile grain: the two heads'
  K=64 score matmuls go to PE row-groups 0/64 concurrently into a double-
  buffered [128, 2, 512] f32 S tile; one exp (free-size 1024) -> es bf16;
  av matmuls ([v_h | ones].T @ es) accumulate y+colsum into [65, 2, 512]
  f32, lagging exp by LAG kv-steps.  Each unit's trailing avs are spread
  over the next unit's first steps; y is evacuated to SBUF early so the
  normalize chain (reciprocal_approx_fast -> gpsimd partition_broadcast ->
  multiply) never blocks PSUM reuse.  PSUM: S 2x2 banks + y 2 banks + a
  2-bank "aux" slot on which projection groups and c_proj t-tiles run as
  small "filler" bundles inside the attention steps, so only 3 projection
  groups precede the exp stream and only the last c_proj tiles follow it.
"""

import sys

sys.path.insert(0, "/opt/trn_rl_repo")

from collections import deque

import numpy as np
import ml_dtypes

import concourse.bacc as bacc
import concourse.bass as bass
import concourse.mybir as mybir
import concourse.tile as tile
from concourse.bass_utils import run_bass_kernel_spmd

F32 = mybir.dt.float32
BF16 = mybir.dt.bfloat16

T = 2048          # sequence length (q and kv)
D = 1024          # model dim
HL = 4            # heads per core
HD = 64           # head dim
DH = HL * HD      # 256 local projected dim
P = 128
QC = 512          # q chunk for attention (y/norm granularity)
NU = T // QC      # 4 q-chunks
NKV = T // P      # 16 kv tiles
SCALE = 1.0 / 8.0  # 1/sqrt(64)
LAG = 10          # kv-steps by which av trails exp

N_CORES = 8

_cache = {}


def build_nc():
    if "nc" in _cache:
        return _cache["nc"]
    nc = bacc.Bacc(
        "TRN2",
        target_bir_lowering=False,
        debug=False,
        num_devices=N_CORES,
    )

    qT = nc.declare_dram_parameter("qT", [D, T], BF16, isOutput=False)
    kT = nc.declare_dram_parameter("kT", [D, T], BF16, isOutput=False)
    vext = nc.declare_dram_parameter(
        "vext", [P, HL, NKV, HD + 1], BF16, isOutput=False
    )
    Wq_r = nc.declare_dram_parameter("Wq_r", [P, 8, DH], BF16, isOutput=False)
    Wk_r = nc.declare_dram_parameter("Wk_r", [P, 8, DH], BF16, isOutput=False)
    Wc_r = nc.declare_dram_parameter("Wc_r", [P, 2, D], BF16, isOutput=False)
    bqk = nc.declare_dram_parameter("bqk", [P, 4], F32, isOutput=False)
    out = nc.declare_dram_parameter("out", [T, D], BF16, isOutput=True)

    with tile.TileContext(nc) as tc:
        with (
            tc.tile_pool(name="wpool", bufs=1) as wpool,
            tc.tile_pool(name="xpool", bufs=16) as xpool,
            tc.tile_pool(name="projsb", bufs=1) as projsb,
            tc.tile_pool(name="espool", bufs=14) as espool,
            tc.tile_pool(name="npool", bufs=2) as npool,
            tc.tile_pool(name="opool", bufs=3) as opool,
            tc.tile_pool(name="ps", bufs=1, space="PSUM") as ps,
        ):
            # ---- weights / constants (host pre-packed, contiguous DMA) ----
            bias_sb = wpool.tile([P, 4], F32, name="bias_sb")  # [bq0,bq1,bk0,bk1]
            nc.sync.dma_start(bias_sb[:], bqk.ap())
            wk_sb = wpool.tile([P, 8, DH], BF16, name="wk_sb")
            nc.sync.dma_start(wk_sb[:], Wk_r.ap())
            wq_sb = wpool.tile([P, 8, DH], BF16, name="wq_sb")
            nc.sync.dma_start(wq_sb[:], Wq_r.ap())
            wc_sb = wpool.tile([P, 2, D], BF16, name="wc_sb")
            ve_sb = wpool.tile([P, HL, NKV, HD + 1], BF16, name="ve_sb")

            # preload the exp activation table during the DMA ramp
            warm_sb = wpool.tile([P, 4], F32, name="warm_sb")
            nc.scalar.activation(
                warm_sb[:], bias_sb[:], mybir.ActivationFunctionType.Exp,
                scale=0.0,
            )

            kpT = projsb.tile([P, 2, T], BF16, name="kpT")
            qpT = projsb.tile([P, 2, T], BF16, name="qpT")
            yallT = projsb.tile([P, 2, T], BF16, name="yallT")

            # ---- input streams, split by column half and ordered so each
            # projection group's operands arrive just-in-time:
            # k cols 0:1024 -> k cols 1024:2048 -> q cols 0:1024 -> rest
            xts = {}

            def xload(nm, xd, tcp):
                for i in range(8):
                    xt = xpool.tile([P, 1024], BF16, tag=f"x{tcp}",
                                    bufs=16, name=f"x{nm}{i}{tcp}")
                    nc.sync.dma_start(
                        xt[:],
                        xd.ap()[i * P:(i + 1) * P,
                                tcp * 1024:(tcp + 1) * 1024],
                    )
                    xts[nm, i, tcp] = xt

            xload("k", kT, 0)
            xload("q", qT, 0)
            xload("k", kT, 1)
            nc.sync.dma_start(ve_sb[:], vext.ap())
            nc.sync.dma_start(wc_sb[:], Wc_r.ap())
            xload("q", qT, 1)

            projw = {"k": (wk_sb, 2, kpT), "q": (wq_sb, 0, qpT)}

            def pgroup_closures(nm, j, tcp, tag="aux", bufs=1):
                """One projection PSUM group = 16 accumulating matmuls +
                bias evac on the aux slot, chopped into <=3-matmul filler
                closures.  Lifecycles on aux are strictly sequential."""
                w_sb, bias_col0, xpT = projw[nm]
                st = {}
                mms = [(i, t2) for i in range(8) for t2 in range(2)]

                def emit(lo, hi):
                    def go():
                        if "g" not in st:
                            st["g"] = ps.tile([P, 2, 512], F32, tag=tag,
                                              bufs=bufs,
                                              name=f"pg{nm}{j}{tcp}")
                        for i, t2 in mms[lo:hi]:
                            nc.tensor.matmul(
                                st["g"][:, t2, :],
                                w_sb[:, i, j * P:(j + 1) * P],
                                xts[nm, i, tcp][:, t2 * 512:(t2 + 1) * 512],
                                start=(i == 0),
                                stop=(i == 7),
                            )
                    return go

                def evac():
                    nc.vector.tensor_tensor(
                        xpT[:, j, tcp * 1024:(tcp + 1) * 1024],
                        st["g"].rearrange("p a b -> p (a b)"),
                        bias_sb[:, bias_col0 + j:bias_col0 + j + 1]
                        .to_broadcast((P, 1024)),
                        mybir.AluOpType.add,
                    )

                cs = [emit(lo, min(lo + 3, 16)) for lo in range(0, 16, 3)]
                cs.append(evac)
                return cs

            def cp_closure(tt, tag="aux", bufs=1):
                """One c_proj t-tile: 4 matmuls (K=256 over j) + evac + DMA,
                as a single small filler closure.  Interleaved (filler) cps
                must stay on the sequential aux slot; the tail cps run
                double-buffered on the "s" slots once attention is done."""
                def go():
                    cp = ps.tile([P, 2, 512], F32, tag=tag, bufs=bufs,
                                 name=f"cp{tt}")
                    for j in range(2):
                        for nch in range(2):
                            nc.tensor.matmul(
                                cp[:, nch, :],
                                yallT[:, j, tt * P:(tt + 1) * P],
                                wc_sb[:, j, nch * 512:(nch + 1) * 512],
                                start=(j == 0),
                                stop=(j == 1),
                            )
                    o_sb = opool.tile([P, D], BF16, tag="osb", name=f"o{tt}")
                    nc.vector.tensor_copy(
                        o_sb[:], cp.rearrange("p a b -> p (a b)")
                    )
                    nc.sync.dma_start(
                        out.ap()[tt * P:(tt + 1) * P, :], o_sb[:]
                    )
                return go

            # inline projection prologue: only what attention unit (0,0)
            # kk 0..7 needs (k j0 cols 0:1024, q j0 cols 0:1024 which covers
            # q-chunks 0 and 1).  The k group runs on the "s" slots (still
            # free) so the q group overlaps it on aux.
            for cl in (pgroup_closures("k", 0, 0, tag="s", bufs=2)
                       + pgroup_closures("q", 0, 0)):
                cl()

            # remaining projection groups stream in as filler, ordered by
            # first-consumer deadline (k01 by unit0 kk=8; k10/q10 by unit1;
            # k11 by unit1 kk=8; q01 by unit4; q11 by unit5)
            filler = deque()
            for nm, j, tcp in (("k", 0, 1), ("k", 1, 0), ("q", 1, 0),
                               ("k", 1, 1), ("q", 0, 1), ("q", 1, 1)):
                filler.extend(pgroup_closures(nm, j, tcp))

            # ---- attention units ----
            def attn_unit(j, c, last=False):
                q0 = c * QC
                st = {}
                es_tiles = {}

                def step_mm(kk):
                    s_ps = ps.tile([P, 2, 512], F32, tag="s", bufs=2,
                                   name=f"s{j}{c}{kk}")
                    for s in range(2):  # head slot: partitions s*64..s*64+64
                        p0 = s * 64
                        nc.tensor.matmul(
                            s_ps[:, s, :],
                            kpT[p0:p0 + HD, j, kk * P:(kk + 1) * P],
                            qpT[p0:p0 + HD, j, q0:q0 + QC],
                            start=True,
                            stop=True,
                        )
                    es = espool.tile([P, 2, QC], BF16, tag="es",
                                     name=f"e{j}{c}{kk}")
                    nc.scalar.activation(
                        es[:], s_ps[:], mybir.ActivationFunctionType.Exp,
                        scale=SCALE,
                    )
                    es_tiles[kk] = es

                def av_mm(kk):
                    if "y" not in st:
                        st["y"] = ps.tile([P, 2, QC], F32, tag="y",
                                          name=f"y{j}{c}")
                    for s in range(2):
                        h = 2 * j + s
                        nc.tensor.matmul(
                            st["y"][0:HD + 1, s, :],
                            ve_sb[:, h, kk, :],
                            es_tiles[kk][:, s, :],
                            start=(kk == 0),
                            stop=(kk == NKV - 1),
                        )
                    del es_tiles[kk]

                def norm():
                    # evacuate y+colsum to SBUF first (frees the PSUM slot),
                    # then normalize off-PSUM.  The final unit's y slot has
                    # no successor, so it skips the y evacuation and its
                    # multiplies read PSUM directly (shorter tail chain).
                    y_ps = st["y"]
                    colsum2 = npool.tile([1, 2, QC], F32, tag="colsum",
                                         name=f"cs{j}{c}")
                    nc.vector.tensor_copy(colsum2[:], y_ps[HD:HD + 1, :, :])
                    if last:
                        yev = y_ps
                    else:
                        yev = npool.tile([HD, 2, QC], F32, tag="yev",
                                         name=f"ye{j}{c}")
                        nc.vector.tensor_copy(yev[:], y_ps[0:HD, :, :])
                    recip2 = npool.tile([1, 2, QC], F32, tag="recip",
                                        name=f"rc{j}{c}")
                    nc.vector.reciprocal_approx_fast(
                        out=recip2[:], in_=colsum2[:]
                    )
                    bcast2 = npool.tile([HD, 2, QC], F32, tag="bcast",
                                        name=f"bc{j}{c}")
                    nc.gpsimd.partition_broadcast(bcast2[:], recip2[:])
                    nc.vector.tensor_tensor(
                        yallT[0:HD, j, q0:q0 + QC],
                        yev[0:HD, 0, :], bcast2[:, 0, :],
                        mybir.AluOpType.mult,
                    )
                    yn = npool.tile([HD, QC], BF16, tag="yn", name=f"yn{j}{c}")
                    nc.vector.tensor_tensor(
                        yn[:], yev[0:HD, 1, :], bcast2[:, 1, :],
                        mybir.AluOpType.mult,
                    )
                    nc.sync.dma_start(
                        yallT[64:64 + HD, j, q0:q0 + QC], yn[:]
                    )

                return step_mm, av_mm, norm

            units = [(j, c) for c in range(NU) for j in range(2)]
            prev = None
            for u_idx, (j, c) in enumerate(units):
                step_mm, av_mm, norm = attn_unit(
                    j, c, last=(u_idx == len(units) - 1)
                )
                for kk in range(NKV):
                    step_mm(kk)
                    if prev is not None:
                        # spread the previous unit's 10 trailing avs over
                        # kk 0..7 (1 each) + the last two with its normalize
                        # at kk=8 — its y-slot is free before av(0) at LAG
                        if kk <= 7 and prev["avs"]:
                            prev["avs"].popleft()()
                        if kk == 8:
                            while prev["avs"]:
                                prev["avs"].popleft()()
                            prev["norm"]()
                            if j == 0 and c >= 1:
                                # q-chunk c-1 rows of yallT complete
                                for tt in range(4 * (c - 1), 4 * c):
                                    filler.append(cp_closure(tt))
                    if kk >= LAG:
                        av_mm(kk - LAG)
                    # last unit: drain avs with extra streams (disjoint
                    # ranges 6..10 and 11..13) so the post-stream tail only
                    # holds avs 14..15 + normalize
                    if u_idx == len(units) - 1 and kk >= 11:
                        av_mm(kk - 5)
                    if u_idx == len(units) - 1 and kk >= 13:
                        av_mm(kk - 2)
                    # pops start at kk=1 so every filler group's last closure
                    # is emitted strictly before its first consumer step;
                    # unit 0 carries no av work before kk=10, so it pops
                    # double from the start to retire k01/k10/q10 early
                    if kk >= 1 and filler:
                        filler.popleft()()
                    if u_idx == 0 and kk >= 1 and filler:
                        filler.popleft()()
                first_trail = 14 if u_idx == len(units) - 1 else NKV - LAG
                prev = {
                    "avs": deque(
                        (lambda kk2=kk2, f=av_mm: f(kk2))
                        for kk2 in range(first_trail, NKV)
                    ),
                    "norm": norm,
                }
            while prev["avs"]:
                prev["avs"].popleft()()
            prev["norm"]()
            for tt in range(4 * (NU - 1), 4 * NU):
                filler.append(cp_closure(tt, tag="s", bufs=2))
            while filler:
                filler.popleft()()

    nc.compile()
    _cache["nc"] = nc
    return nc


def make_in_maps(k, q, v, Wk, bk, Wq, bq, Wc, bc):
    bf = ml_dtypes.bfloat16
    k = np.asarray(k, dtype=np.float32)
    q = np.asarray(q, dtype=np.float32)
    v = np.asarray(v, dtype=np.float32)
    Wk = np.asarray(Wk, dtype=np.float32)
    Wq = np.asarray(Wq, dtype=np.float32)
    Wc = np.asarray(Wc, dtype=np.float32)
    bk = np.asarray(bk, dtype=np.float32)
    bq = np.asarray(bq, dtype=np.float32)
    in_maps = []
    for cidx in range(N_CORES):
        b = cidx // 4
        h0 = (cidx % 4) * HL
        sl = slice(h0 * HD, h0 * HD + DH)
        bq_t = np.ascontiguousarray(bq[sl].reshape(2, P).T)  # [128, 2]
        bk_t = np.ascontiguousarray(bk[sl].reshape(2, P).T)
        bqk = np.concatenate([bq_t, bk_t], axis=1)           # [128, 4]
        # vext [P, HL, NKV, HD+1]: [p, h, m, d] = v[m*128+p, sl][h*64+d],
        # ones at d=64 (colsum row for the av matmul)
        vsl = v[b][:, sl]                                    # [T, 256]
        ve = np.ones((P, HL, NKV, HD + 1), dtype=np.float32)
        ve[:, :, :, 0:HD] = (
            vsl.reshape(NKV, P, HL, HD).transpose(1, 2, 0, 3)
        )
        # Wq_r [128, 8, 256]: [p, i, m] = Wq[sl,:].T[i*128+p, m]
        wq_t = Wq[sl, :].T.reshape(8, P, DH).transpose(1, 0, 2)
        wk_t = Wk[sl, :].T.reshape(8, P, DH).transpose(1, 0, 2)
        wc_t = Wc[:, sl].T.reshape(2, P, D).transpose(1, 0, 2)
        in_maps.append({
            "qT": np.ascontiguousarray(q[b].T).astype(bf),
            "kT": np.ascontiguousarray(k[b].T).astype(bf),
            "vext": np.ascontiguousarray(ve).astype(bf),
            "Wq_r": np.ascontiguousarray(wq_t).astype(bf),
            "Wk_r": np.ascontiguousarray(wk_t).astype(bf),
            "Wc_r": np.ascontiguousarray(wc_t).astype(bf),
            "bqk": np.ascontiguousarray(bqk),
        })
    return in_maps


def kernel(k, q, v, Wk, bk, Wq, bq, Wc, bc, _trace=False, _trace_cores=None):
    bc = np.asarray(bc, dtype=np.float32)
    nc = build_nc()
    in_maps = make_in_maps(k, q, v, Wk, bk, Wq, bq, Wc, bc)
    res = run_bass_kernel_spmd(
        nc, in_maps, core_ids=list(range(N_CORES)),
        trace=_trace, trace_cores=_trace_cores,
    )
    outs = [res.results[c]["out"].astype(np.float32) for c in range(N_CORES)]
    full = np.stack([
        outs[0] + outs[1] + outs[2] + outs[3],
        outs[4] + outs[5] + outs[6] + outs[7],
    ]) + bc[None, None, :]
    kernel.last_result = res
    return full.astype(np.float32)
